# revision 18
# baseline (speedup 1.0000x reference)
"""Bass/Tile TRN2 kernel for nn_ExpressionAttentionLayer.

Math per batch b (B=8, G=2048, D=64):
    K_fused = concat([K_gene, K_expr], -1) @ WK_w.T + WK_b      # (G, D)
    Q_fused = concat([Q_gene, Q_expr], -1) @ WQ_w.T + WQ_b      # (G, D)
    A       = softmax(Q_fused @ K_fused.T / sqrt(D), axis=-1)
    out     = (A * M) @ V_expr                                   # (G, D)

Sharding: data-parallel over batch; core i handles batch i (B == n_cores == 8).
No collectives.

Per-core dataflow (v4):
  - PE clock (HAM): the core boots at 1.2 GHz and is only granted 2.4 GHz
    after a long window of sustained tensor-engine utilization; any idle
    gap delays the grant.  A WAW-chained junk-matmul bridge starts the
    instant the engine preamble ends and runs until the first real
    transposes, so PE utilization is ~100% from t~6us.
  - Queue split so the M stream owns the SWDGE queue almost immediately:
      sync   ring : K_gene, biases, Q_gene, M fp32 tail-slices (4 tiles/DMA)
      scalar ring : K_expr, Q_expr, V (natural chunk layout), out-writes
      SWDGE queue : WK_w, WQ_w (bf16 cast), then M cols 0:1536 as bf16
                    cast half-tiles, tile after tile, no competitors.
  - M per q-tile: cols 0:1536 arrive bf16 via SWDGE (2 DMAs), cols
    1536:2048 arrive fp32 via the sync ring in groups of 4 tiles (1MB
    DMAs); the em multiply for that slice runs in DVE 1x mixed mode.
    Splitting M across two queues lifts aggregate M bandwidth above the
    single-SWDGE-queue ceiling (~370 GB/s observed).
  - Q/K loads use "(p s)" partition-contiguous DRAM layouts; V loads
    "(s p)" so chunk kt lands directly at [128, kt, 64] (no on-chip
    redistribute).  PE transposes run on bf16 (1 cycle/row).
  - K side feeds kfT first (gates every logits matmul); K transposed
    chunks scatter through stride-16 views to natural k order.  Q keeps
    chunk order; the M/out DRAM views absorb the q permutation.
  - Loop: pair (2t, 2t+1); logits row-packed into PE halves via the
    duplicated qfT/kfT partition halves; exp on ACT with fp32 row-sum
    accum (scale folded; no max subtraction needed, |logits| small);
    em = ex*M on DVE; PE-transpose em; AV accumulates over k on PE;
    softmax reciprocal applied on ACT while leaving PSUM; out on the
    scalar ring.  The previous pair's AV fills the h-gap; the final
    pair's AV is emitted inline so the tail doesn't serialize.
"""

from contextlib import ExitStack

import numpy as np

import concourse.bass as bass
import concourse.tile as tile
from concourse import bacc, mybir
from concourse.bass_utils import run_bass_kernel_spmd

B, G, D = 8, 2048, 64
P = 128
NT = G // P  # 16 tiles of 128 rows
F32 = mybir.dt.float32
BF16 = mybir.dt.bfloat16
AF = mybir.ActivationFunctionType
MS = 1536  # M columns carried by the SWDGE bf16 stream; rest on sync ring fp32

N_CORES = 8


def _emit(ctx: ExitStack, tc: tile.TileContext, io: dict):
    nc = tc.nc

    singles = ctx.enter_context(tc.tile_pool(name="singles", bufs=1))
    ld = ctx.enter_context(tc.tile_pool(name="ld", bufs=4))

    # PSUM pools (8 banks total: ps_l 2x2 + ps_t 2x1 + ps_o 2x1 = 8)
    ps_l = ctx.enter_context(tc.tile_pool(name="ps_l", bufs=2, space="PSUM"))
    ps_t = ctx.enter_context(tc.tile_pool(name="ps_t", bufs=2, space="PSUM"))
    ps_o = ctx.enter_context(tc.tile_pool(name="ps_o", bufs=2, space="PSUM"))

    # ---- HAM warmup bridge: WAW-chained junk matmuls, first thing on the
    # PE queue.  Accumulating into one psum tile keeps them back-to-back
    # (in-order engine queue, no cross-engine semaphore round-trips).
    junk = singles.tile([P, 512], BF16, tag="junk")
    nc.vector.memset(junk[:], 0.0)
    ps_junk = ps_o.tile([P, 512], F32, tag="ps_o", name="ps_warm")
    NJUNK = 9
    for i in range(NJUNK):
        nc.tensor.matmul(
            ps_junk[:], junk[:, 0:P], junk[:], start=(i == 0), stop=(i == NJUNK - 1)
        )

    # identity directly in bf16 at the HEAD of the gpsimd queue (before the
    # SWDGE loads), so it's ready long before the first transposes.
    identity_bf = singles.tile([P, P], BF16, tag="identity_bf")
    nc.gpsimd.memset(identity_bf[:], 0.0)
    nc.gpsimd.affine_select(
        out=identity_bf[:],
        in_=identity_bf[:],
        compare_op=mybir.AluOpType.not_equal,
        fill=1.0,
        base=0,
        pattern=[[-1, P]],
        channel_multiplier=1,
    )

    # ---- M-tile streams ----
    # Row-permuted view: iteration qt covers logical q rows {p*16 + qt},
    # matching the Q-side chunk order.  SWDGE carries cols 0:MS as bf16
    # cast half-tiles; the sync ring carries cols MS:2048 fp32 in groups
    # of 4 tiles (1MB DMAs, 2KB descriptors).
    mpool = ctx.enter_context(tc.tile_pool(name="mpool", bufs=6))
    mspool = ctx.enter_context(tc.tile_pool(name="mspool", bufs=3))
    m_r = io["M"].rearrange("(p s) k -> s p k", s=NT)
    mts = {}
    msl = {}

    def issue_m(qt):
        if qt < NT:
            mt = mpool.tile([P, MS], BF16, tag="m", name="m")
            nc.gpsimd.dma_start(mt[:, 0:1024], m_r[qt, :, 0:1024])
            nc.gpsimd.dma_start(mt[:, 1024:MS], m_r[qt, :, 1024:MS])
            mts[qt] = mt

    def issue_mslice(g):
        # group g = pair g: fp32 tail-slices for tiles 2g, 2g+1 (one 0.5MB DMA).
        # SBUF dst AP must be partition-first; reorder the DRAM side instead.
        if 2 * g < NT:
            ms = mspool.tile([P, 2, G - MS], F32, tag="ms", name="ms")
            nc.sync.dma_start(
                ms[:], m_r[2 * g : 2 * g + 2, :, MS:G].rearrange("s p k -> p s k")
            )
            msl[g] = ms

    # ---- weights as SWDGE bf16 cast-loads (first on the gpsimd queue —
    # tiny, then the queue belongs to the M stream).
    wk_nat = singles.tile([D, 2 * D], BF16, tag="wk_nat")
    wq_nat = singles.tile([D, 2 * D], BF16, tag="wq_nat")
    nc.gpsimd.dma_start(wk_nat[:], io["WK_w"][:, :])
    nc.gpsimd.dma_start(wq_nat[:], io["WQ_w"][:, :])

    # SWDGE: the whole M stream queues here, right behind the weights.
    for _qt in range(6):
        issue_m(_qt)

    # ---- HWDGE input loads ----
    bigs = {}
    bigs_bf = {}
    # sync ring: K_gene, biases, Q_gene, then M slices.
    big = ld.tile([P, NT, D], F32, tag="ld_K_gene", name="ld_K_gene")
    nc.sync.dma_start(big[:], io["K_gene"].rearrange("(p s) d -> p s d", s=NT))
    bigs["K_gene"] = big
    # biases: one DMA each onto partition half 0 (DVE dup emitted later so
    # it can't head-of-line-block the K casts on the in-order DVE queue)
    wkb = singles.tile([P, 1], F32, tag="wkb")
    wqb = singles.tile([P, 1], F32, tag="wqb")
    nc.sync.dma_start(wkb[0:D], io["WK_b"][:, None])
    nc.sync.dma_start(wqb[0:D], io["WQ_b"][:, None])
    big = ld.tile([P, NT, D], F32, tag="ld_Q_gene", name="ld_Q_gene")
    nc.sync.dma_start(big[:], io["Q_gene"].rearrange("(p s) d -> p s d", s=NT))
    bigs["Q_gene"] = big
    issue_mslice(0)
    issue_mslice(1)
    issue_mslice(2)

    # scalar ring: K_expr, Q_expr, V (natural chunk layout), then out-writes.
    big = ld.tile([P, NT, D], F32, tag="ld_K_expr", name="ld_K_expr")
    nc.scalar.dma_start(big[:], io["K_expr"].rearrange("(p s) d -> p s d", s=NT))
    bigs["K_expr"] = big
    big = ld.tile([P, NT, D], F32, tag="ld_Q_expr", name="ld_Q_expr")
    nc.scalar.dma_start(big[:], io["Q_expr"].rearrange("(p s) d -> p s d", s=NT))
    bigs["Q_expr"] = big
    # V in "(s p)" layout: chunk kt lands at [128, kt, 64] directly.
    v_f32 = ld.tile([P, NT, D], F32, tag="v_f32", name="v_f32")
    nc.scalar.dma_start(v_f32[:], io["V_expr"].rearrange("(s p) d -> p s d", p=P))
    v_bf = singles.tile([P, NT, D], BF16, tag="v_bf")

    # ---- input casts to bf16 (PE transposes are 1-pass on bf16) ----
    # K_gene/Q_gene on DVE, K_expr/Q_expr on ACT, half-split so the first
    # transposes start as early as possible.  These are emitted FIRST on
    # their engine queues (before any other dependent work) because the
    # queues are in-order and K gates the whole pipeline.
    for src_name in ("K_gene", "K_expr", "Q_gene", "Q_expr"):
        bigs_bf[src_name] = ld.tile(
            [P, NT, D], BF16, tag=f"ldb_{src_name}", name=f"ldb_{src_name}"
        )
    for half in range(2):
        sl = (slice(None), slice(8 * half, 8 * half + 8), slice(None))
        nc.vector.tensor_copy(bigs_bf["K_gene"][sl], bigs["K_gene"][sl])
        nc.scalar.copy(bigs_bf["K_expr"][sl], bigs["K_expr"][sl])
    for half in range(2):
        sl = (slice(None), slice(8 * half, 8 * half + 8), slice(None))
        nc.vector.tensor_copy(bigs_bf["Q_gene"][sl], bigs["Q_gene"][sl])
        nc.scalar.copy(bigs_bf["Q_expr"][sl], bigs["Q_expr"][sl])

    # ---- weight transposes: PE-transpose the two [64,64] halves in bf16 ----
    wk_gTb = singles.tile([D, D], BF16, tag="wk_gTb")
    wk_eTb = singles.tile([D, D], BF16, tag="wk_eTb")
    wq_gTb = singles.tile([D, D], BF16, tag="wq_gTb")
    wq_eTb = singles.tile([D, D], BF16, tag="wq_eTb")
    for nat, dsts in ((wk_nat, (wk_gTb, wk_eTb)), (wq_nat, (wq_gTb, wq_eTb))):
        for h, dst in enumerate(dsts):
            psw = ps_t.tile([P, 8 * P], BF16, tag="ps_t", name="ps_w")[:D, :D]
            nc.tensor.transpose(
                psw[:], nat[:, h * D : (h + 1) * D], identity_bf[:D, :D]
            )
            nc.vector.tensor_copy(dst[:], psw[:])
    # bias dup onto partition half 1 (needed by the projections ~11.5us)
    nc.vector.tensor_copy(wkb[D : 2 * D], wkb[0:D])
    nc.vector.tensor_copy(wqb[D : 2 * D], wqb[0:D])

    # ---- transpose K/Q gene+expr into bf16 [D, G] (d on partitions) ----
    kgT = singles.tile([D, G], BF16, tag="kgT")
    keT = singles.tile([D, G], BF16, tag="keT")
    qgT = singles.tile([D, G], BF16, tag="qgT")
    qeT = singles.tile([D, G], BF16, tag="qeT")
    # fused tensors duplicated on BOTH partition halves for row-packing
    kfT = singles.tile([P, G], BF16, tag="kfT")
    qfT = singles.tile([P, G], BF16, tag="qfT")

    def emit_transposes(side, gT, eT, j):
        # Chunk order: chunk s -> columns [s*128,(s+1)*128), g = p*16 + s.
        if side == "K":
            engines = ((0, gT, nc.vector), (1, eT, nc.vector))
        else:
            engines = ((0, gT, nc.vector), (1, eT, nc.scalar))
        for c, dstT, ceng in engines:
            big = bigs_bf[f"{side}_gene" if c == 0 else f"{side}_expr"]
            ps = ps_t.tile([P, 8 * P], BF16, tag="ps_t", name="ps_tr")[:D, : 4 * P]
            for i in range(4):
                s = 4 * j + i
                nc.tensor.transpose(
                    ps[:, i * P : (i + 1) * P], big[:, s, :], identity_bf[:]
                )
            dst = dstT[:, j * 512 : (j + 1) * 512].rearrange("d (i p) -> d i p", i=4)
            src = ps[:].rearrange("d (i p) -> d i p", i=4)
            if ceng is nc.vector:
                ceng.tensor_copy(dst, src)
            else:
                ceng.copy(dst, src)

    def emit_proj(gT, eT, wgT, weT, b_sb, fT, j, permute):
        # K side: chunk->natural column reorder rides the moving-operand AP.
        if permute:
            rg = gT[:].rearrange("d (s p) -> d p s", s=NT)[:, j * 32 : (j + 1) * 32, :]
            re = eT[:].rearrange("d (s p) -> d p s", s=NT)[:, j * 32 : (j + 1) * 32, :]
        else:
            rg = gT[:, j * 512 : (j + 1) * 512]
            re = eT[:, j * 512 : (j + 1) * 512]
        psj = ps_o.tile([P, 512], F32, tag="ps_o", name="ps_pj")
        for half in range(2):
            hsl = slice(half * D, (half + 1) * D)
            nc.tensor.matmul(psj[hsl, :], wgT[:], rg, start=True, stop=False)
            nc.tensor.matmul(psj[hsl, :], weT[:], re, start=False, stop=True)
        if permute:
            nc.vector.tensor_scalar_add(
                fT[:, j * 512 : (j + 1) * 512], psj[:], b_sb[:, 0:1]
            )
        else:
            nc.scalar.activation(
                fT[:, j * 512 : (j + 1) * 512], psj[:], AF.Identity, bias=b_sb[:, 0:1]
            )

    for j in range(4):
        emit_transposes("K", kgT, keT, j)
    for j in range(4):
        emit_proj(kgT, keT, wk_gTb, wk_eTb, wkb, kfT, j, permute=True)
    # Q block 0 only — blocks 1-3 ride the loop's PE slack.
    emit_transposes("Q", qgT, qeT, 0)
    emit_proj(qgT, qeT, wq_gTb, wq_eTb, wqb, qfT, 0, permute=False)
    # V cast here on DVE: after everything that gates the loop start, but
    # before the first AV needs v_bf (~pair 0's AV).
    nc.vector.tensor_copy(v_bf[:], v_f32[:])

    def emit_q_block(j):
        emit_transposes("Q", qgT, qeT, j)
        emit_proj(qgT, qeT, wq_gTb, wq_eTb, wqb, qfT, j, permute=False)

    # ---- main attention loop (fully per-q-tile pipelined) ----
    epool = ctx.enter_context(tc.tile_pool(name="epool", bufs=4))
    empool = ctx.enter_context(tc.tile_pool(name="empool", bufs=4))
    tpool = ctx.enter_context(tc.tile_pool(name="tpool", bufs=4))
    opool = ctx.enter_context(tc.tile_pool(name="opool", bufs=4))
    rspool = ctx.enter_context(tc.tile_pool(name="rspool", bufs=4))

    out_r = io["out"].rearrange("(p s) d -> s p d", s=NT)
    scale = 1.0 / np.sqrt(np.float32(D))

    pending = []  # [(qt, emt, recip), ...]

    def emit_av(pend):
        qt_p, emt_p, recip_p = pend
        pso = ps_o.tile([P, 512], F32, tag="ps_o", name="ps_av")[:, :D]
        for kt in range(NT):
            nc.tensor.matmul(
                pso[:],
                emt_p[:, kt, :],
                v_bf[:, kt, :],
                start=(kt == 0),
                stop=(kt == NT - 1),
            )
        ob = opool.tile([P, D], F32, tag="ob")
        nc.scalar.activation(ob[:], pso[:], AF.Copy, bias=0.0, scale=recip_p[:, 0:1])
        nc.scalar.dma_start(out_r[qt_p], ob[:])

    for t in range(NT // 2):
        qts = (2 * t, 2 * t + 1)
        mtab = [mts.pop(qt) for qt in qts]
        mst = msl.pop(t)
        issue_m(2 * t + 6)
        issue_m(2 * t + 7)
        issue_mslice(t + 3)

        emtab = [tpool.tile([P, NT, P], BF16, tag="emt", name="emt") for _ in range(2)]
        rsab = [
            [rspool.tile([P, 1], F32, tag=f"rs{a}{h}", name=f"rs{a}{h}") for h in range(2)]
            for a in range(2)
        ]

        for h in range(2):
            hsl = slice(h * 1024, (h + 1) * 1024)
            psls = []
            for a in range(2):
                # row-packed logits: lhsT/rhs from partition half a
                psl = ps_l.tile([P, 1024], F32, tag="ps_l")
                asl = slice(a * D, (a + 1) * D)
                for n in range(2):
                    nc.tensor.matmul(
                        psl[:, n * 512 : (n + 1) * 512],
                        qfT[asl, qts[a] * P : (qts[a] + 1) * P],
                        kfT[asl, (2 * h + n) * 512 : (2 * h + n + 1) * 512],
                        start=True,
                        stop=True,
                    )
                psls.append(psl)
            exh = [epool.tile([P, 1024], BF16, tag="ex", name="ex") for _ in range(2)]
            emh = [empool.tile([P, 1024], BF16, tag="em", name="em") for _ in range(2)]
            for a in range(2):
                # exp -> bf16 with fp32 row-sum accumulation
                nc.scalar.activation(
                    exh[a][:],
                    psls[a][:],
                    AF.Exp,
                    scale=float(scale),
                    accum_out=rsab[a][h][:],
                )
                # ex * M: bf16 2x for the SWDGE cols, mixed 1x for the
                # fp32 sync-ring slice
                if h == 0:
                    nc.vector.tensor_mul(emh[a][:], exh[a][:], mtab[a][:, 0:1024])
                else:
                    nc.vector.tensor_mul(
                        emh[a][:, 0 : MS - 1024],
                        exh[a][:, 0 : MS - 1024],
                        mtab[a][:, 1024:MS],
                    )
                    nc.vector.tensor_mul(
                        emh[a][:, MS - 1024 : 1024],
                        exh[a][:, MS - 1024 : 1024],
                        mst[:, a, :],
                    )

            # previous pair's AV runs on PE between this pair's halves
            if h == 0:
                for pend in pending:
                    emit_av(pend)
                pending = []

            for a in range(2):
                # transpose this half's 8 [128,128] blocks; 2x DVE copy out
                pst = ps_t.tile([P, 8 * P], BF16, tag="ps_t")
                for k in range(8):
                    nc.tensor.transpose(
                        pst[:, k * P : (k + 1) * P],
                        emh[a][:, k * P : (k + 1) * P],
                        identity_bf[:],
                    )
                nc.vector.tensor_copy(
                    emtab[a][:, 8 * h : 8 * h + 8, :],
                    pst[:].rearrange("p (a b) -> p a b", a=8),
                )

        for a in range(2):
            rsum = rspool.tile([P, 1], F32, tag=f"rsum{a}", name="rsum")
            nc.vector.tensor_add(rsum[:], rsab[a][0][:], rsab[a][1][:])
            recip = rspool.tile([P, 1], F32, tag=f"recip{a}", name="recip")
            nc.vector.reciprocal(recip[:], rsum[:])
            pending.append((qts[a], emtab[a], recip))

        # deferred Q-side blocks ride the loop's PE slack
        if t < 3:
            emit_q_block(t + 1)

        # final pair: AV inline so the tail doesn't serialize behind the loop
        if t == NT // 2 - 1:
            for pend in pending:
                emit_av(pend)
            pending = []

    for pend in pending:
        emit_av(pend)


def _build():
    # Bacc (not plain Bass): its compile() legalizes sync waits which
    # walrus codegen requires (max 1 wait per instruction).
    nc = bacc.Bacc("TRN2", target_bir_lowering=False, debug=False)
    io = {}
    for name in ("Q_gene", "K_gene", "Q_expr", "K_expr", "V_expr"):
        io[name] = nc.dram_tensor(name, [G, D], F32, kind="ExternalInput").ap()
    io["M"] = nc.dram_tensor("M", [G, G], F32, kind="ExternalInput").ap()
    for name in ("WK_w", "WQ_w"):
        io[name] = nc.dram_tensor(name, [D, 2 * D], F32, kind="ExternalInput").ap()
    for name in ("WK_b", "WQ_b"):
        io[name] = nc.dram_tensor(name, [D], F32, kind="ExternalInput").ap()
    io["out"] = nc.dram_tensor("out", [G, D], F32, kind="ExternalOutput").ap()

    with tile.TileContext(nc) as tc:
        with ExitStack() as ctx:
            _emit(ctx, tc, io)
    nc.compile()
    return nc


_NC = None


def _get_nc():
    global _NC
    if _NC is None:
        _NC = _build()
    return _NC


def kernel(**inputs) -> np.ndarray:
    return run_kernel_with_results(**inputs)[0]


def run_kernel_with_results(trace=False, **inputs):
    """Returns (full_output, BassKernelResults)."""
    nc = _get_nc()
    per_core_names = ("Q_gene", "K_gene", "Q_expr", "K_expr", "V_expr", "M")
    shared_names = ("WK_w", "WK_b", "WQ_w", "WQ_b")
    arrs = {k: np.ascontiguousarray(np.asarray(v), dtype=np.float32) for k, v in inputs.items()}
    in_maps = []
    for c in range(N_CORES):
        im = {n: arrs[n][c] for n in per_core_names}
        for n in shared_names:
            im[n] = arrs[n]
        in_maps.append(im)
    res = run_bass_kernel_spmd(nc, in_maps, list(range(N_CORES)), trace=trace)
    out = np.stack([res.results[c]["out"] for c in range(N_CORES)], axis=0)
    return out.astype(np.float32), res
